# revision 20
# baseline (speedup 1.0000x reference)
"""BiMambaBlock on 8 Trainium2 NeuronCores.

Sharding: core c = (batch b, direction d, d_inner-half h) with
b = c>>2, d = (c>>1)&1, h = c&1.  Every core runs the same program on
different data (weights sliced/permuted per core on the host):

  - host feeds x[b].T in bf16 (flipped along L for bwd cores, padded
    with 3 leading zero cols for the causal conv), so the device always
    runs a *forward* mamba mixer in channels-on-partitions layout [d, L].
  - each core computes the full xc = silu(conv(x @ in_w_xi)) over all
    1024 channels (so the xproj contraction over d_inner stays local,
    no collectives), but scans only its 512-channel half (the host
    permutes weights so the own half is always channel blocks 0-3).
  - out_proj and the final fuse matmul are folded on the host into one
    [512ch, 512dm] weight; each core emits a partial [512dm, L] f32
    which the host transposes/flips/sums.

Everything except PSUM accumulation runs in bf16 (the scan keeps fp32
internal state).  The full L=2048 is processed unchunked: the
tensor_tensor_scan has a ~2.5us fixed cost, so one long scan per
(block, n) beats four chained chunk scans.  B_n / C_n rows are
broadcast across partitions with PE selector matmuls (lhsT one-hot
columns), drained to bf16 so the dBx / C-mul tensor_tensor ops hit the
DVE 2x bf16 mode.  The y += h*C accumulation chain runs on the
otherwise-idle GPSIMD engine.
"""
import os
import sys

for _p in ("/opt/trn_rl_repo",):
    if os.path.isdir(_p) and _p not in sys.path:
        sys.path.insert(0, _p)

from contextlib import ExitStack

import ml_dtypes
import numpy as np

from concourse import bass, mybir, tile
from concourse.bass_utils import run_bass_kernel_spmd

F32 = mybir.dt.float32
BF16 = mybir.dt.bfloat16
AF = mybir.ActivationFunctionType
OP = mybir.AluOpType

D_MODEL = 512
D_INNER = 1024
DH = 512
N_STATE = 16
D_CONV = 4
DT_RANK = 32
B = 2
L = 2048
LP = L + 3

NBLK_F = D_INNER // 128  # 8 channel blocks for conv/xproj
NBLK_H = DH // 128       # 4 scan blocks

NCH = L // 512           # 512-wide matmul N-chunks

BF16NP = ml_dtypes.bfloat16


def _build_program():
    nc = bass.Bass(trn_type="TRN2", target_bir_lowering=False, debug=False)

    xT_d = nc.dram_tensor("xT", [128, 4 * LP], BF16, kind="ExternalInput")
    w_in_d = nc.dram_tensor("w_in", [128, 4 * 1536], BF16, kind="ExternalInput")
    conv_w_d = nc.dram_tensor("conv_w", [128, NBLK_F * D_CONV], F32, kind="ExternalInput")
    conv_b_d = nc.dram_tensor("conv_b", [128, NBLK_F], F32, kind="ExternalInput")
    xproj_w_d = nc.dram_tensor("xproj_w", [128, NBLK_F * 64], BF16, kind="ExternalInput")
    dt_w_d = nc.dram_tensor("dt_w", [DT_RANK, DH], BF16, kind="ExternalInput")
    dt_b_d = nc.dram_tensor("dt_b", [128, NBLK_H], F32, kind="ExternalInput")
    A_d = nc.dram_tensor("A", [128, NBLK_H * N_STATE], F32, kind="ExternalInput")
    D_d = nc.dram_tensor("D", [128, NBLK_H], F32, kind="ExternalInput")
    w_out_d = nc.dram_tensor("w_out", [128, 4 * D_MODEL], BF16, kind="ExternalInput")
    bcsel_d = nc.dram_tensor("bcsel", [N_STATE, N_STATE * 128], BF16,
                             kind="ExternalInput")
    out_d = nc.dram_tensor("out_part", [D_MODEL, L], F32, kind="ExternalOutput")

    with tile.TileContext(nc) as tc, ExitStack() as ctx:
        # ---------------- global pools / persistent tiles ----------------
        wp = ctx.enter_context(tc.tile_pool(name="weights", bufs=1))

        xT = wp.tile([128, 4 * LP], BF16, tag="xT")
        w_in = wp.tile([128, 4 * 1536], BF16, tag="w_in")
        conv_w = wp.tile([128, NBLK_F * D_CONV], F32, tag="conv_w")
        conv_b = wp.tile([128, NBLK_F], F32, tag="conv_b")
        xproj_w = wp.tile([128, NBLK_F * 64], BF16, tag="xproj_w")
        dt_w = wp.tile([DT_RANK, DH], BF16, tag="dt_w")
        dt_b = wp.tile([128, NBLK_H], F32, tag="dt_b")
        A_sb = wp.tile([128, NBLK_H * N_STATE], F32, tag="A")
        D_sb = wp.tile([128, NBLK_H], F32, tag="D")
        w_out = wp.tile([128, 4 * D_MODEL], BF16, tag="w_out")
        bcsel = wp.tile([N_STATE, N_STATE * 128], BF16, tag="bcsel")

        for t, d in [(xT, xT_d), (w_in, w_in_d), (conv_w, conv_w_d),
                     (conv_b, conv_b_d), (xproj_w, xproj_w_d), (dt_w, dt_w_d),
                     (dt_b, dt_b_d), (A_sb, A_d), (D_sb, D_d), (w_out, w_out_d),
                     (bcsel, bcsel_d)]:
            nc.sync.dma_start(t[:], d[:])

        xT_v = xT[:].rearrange("p (k l) -> p k l", k=4)
        w_in_v = w_in[:].rearrange("p (k m) -> p k m", k=4)
        xproj_v = xproj_w[:].rearrange("p (k f) -> p k f", k=NBLK_F)
        w_out_v = w_out[:].rearrange("p (k m) -> p k m", k=4)

        pio = ctx.enter_context(tc.tile_pool(name="pio", bufs=4, space="PSUM"))
        pdbc = ctx.enter_context(tc.tile_pool(name="pdbc", bufs=2, space="PSUM"))

        glob = ctx.enter_context(tc.tile_pool(name="glob", bufs=1))
        xc_t = [glob.tile([128, L], BF16, tag=f"xc{i}", name=f"xc{i}")
                for i in range(NBLK_H)]  # own-half xc, live till the end
        dt_t = [glob.tile([128, L], BF16, tag=f"dt{i}", name=f"dt{i}")
                for i in range(NBLK_H)]
        dtx_t = [glob.tile([128, L], BF16, tag=f"dtx{i}", name=f"dtx{i}")
                 for i in range(NBLK_H)]
        y_t = [glob.tile([128, L], BF16, tag=f"y{i}", name=f"y{i}")
               for i in range(NBLK_H)]
        dbc = glob.tile([64, L], BF16, tag="dbc")
        B_sb = glob.tile([N_STATE, L], BF16, tag="Brows")
        C_sb = glob.tile([N_STATE, L], BF16, tag="Crows")

        def in_proj_block(m0, xi, xi_off, n_cols):
            """matmul w_in cols [m0, m0+128) x xT -> xi[:, xi_off:...]"""
            for nch in range(0, n_cols, 512):
                w = min(512, n_cols - nch)
                ps = pio.tile([128, 512], F32, tag="pio", name="ps_in")
                for kb in range(4):
                    nc.tensor.matmul(
                        ps[:, 0:w],
                        lhsT=w_in_v[:, kb, m0:m0 + 128],
                        rhs=xT_v[:, kb, nch:nch + w],
                        start=(kb == 0), stop=(kb == 3),
                    )
                nc.scalar.copy(xi[:, xi_off + nch:xi_off + nch + w], ps[:, 0:w])

        # ---------------- phase 1: xc / xproj / dt ----------------
        with tc.tile_pool(name="ph1", bufs=1) as ph1, \
             tc.tile_pool(name="ph1b", bufs=2) as ph1b:
            for blk in range(NBLK_F):
                xi = ph1b.tile([128, LP], BF16, tag="xi", name="xi")
                in_proj_block(blk * 128, xi, 0, LP)
                acc = ph1b.tile([128, L], BF16, tag="acc", name="acc")
                nc.vector.tensor_scalar_mul(
                    acc[:], xi[:, 0:L], conv_w[:, blk * 4:blk * 4 + 1])
                for k in range(1, D_CONV):
                    nc.vector.scalar_tensor_tensor(
                        acc[:], xi[:, k:k + L],
                        conv_w[:, blk * 4 + k:blk * 4 + k + 1], acc[:],
                        OP.mult, OP.add)
                if blk < NBLK_H:
                    xc = xc_t[blk]
                else:
                    xc = ph1.tile([128, L], BF16, tag=f"xcO{blk}",
                                  name=f"xcO{blk}")
                nc.scalar.activation(xc[:], acc[:], AF.Silu,
                                     bias=conv_b[:, blk:blk + 1])
                if blk < NBLK_H:
                    xc_t[blk] = xc
                else:
                    xc_t.append(xc)

            # xproj -> dbc.T [64, L]
            for nch in range(NCH):
                ps = pdbc.tile([64, 512], F32, tag="pdbc", name="ps_dbc")
                for kb in range(NBLK_F):
                    nc.tensor.matmul(
                        ps[:], lhsT=xproj_v[:, kb, :],
                        rhs=xc_t[kb][:, nch * 512:(nch + 1) * 512],
                        start=(kb == 0), stop=(kb == NBLK_F - 1),
                    )
                nc.scalar.copy(dbc[:, nch * 512:(nch + 1) * 512], ps[:])

            # dt = softplus(dt_raw.T + dt_b) = ln(1 + exp(.))
            for m in range(NBLK_H):
                dte = ph1b.tile([128, L], BF16, tag="dte", name="dte")
                for nch in range(NCH):
                    ps = pio.tile([128, 512], F32, tag="pio", name="ps_dt")
                    nc.tensor.matmul(
                        ps[:], lhsT=dt_w[:, m * 128:(m + 1) * 128],
                        rhs=dbc[0:DT_RANK, nch * 512:(nch + 1) * 512],
                        start=True, stop=True)
                    nc.scalar.activation(dte[:, nch * 512:(nch + 1) * 512],
                                         ps[:], AF.Exp, bias=dt_b[:, m:m + 1])
                nc.scalar.activation(dt_t[m][:], dte[:], AF.Ln, bias=1.0)
                nc.vector.tensor_tensor(dtx_t[m][:], dt_t[m][:], xc_t[m][:],
                                        OP.mult)

            nc.sync.dma_start(B_sb[:], dbc[32:48, :])
            nc.sync.dma_start(C_sb[:], dbc[48:64, :])

        # ---------------- phase 2: scan over (n, blk) ----------------
        with tc.tile_pool(name="ph2", bufs=2) as ph2:
            for n in range(N_STATE):
                Bt = ph2.tile([128, L], BF16, tag="Bt", name="Bt")
                Ct = ph2.tile([128, L], BF16, tag="Ct", name="Ct")
                for src, dst in ((B_sb, Bt), (C_sb, Ct)):
                    for nch in range(NCH):
                        ps = pio.tile([128, 512], F32, tag="pio", name="ps_bc")
                        nc.tensor.matmul(
                            ps[:], lhsT=bcsel[:, n * 128:(n + 1) * 128],
                            rhs=src[:, nch * 512:(nch + 1) * 512],
                            start=True, stop=True)
                        nc.scalar.copy(dst[:, nch * 512:(nch + 1) * 512], ps[:])
                for blk in range(NBLK_H):
                    dA = ph2.tile([128, L], BF16, tag="dA", name="dA")
                    nc.scalar.activation(
                        dA[:], dt_t[blk][:], AF.Exp,
                        scale=A_sb[:, blk * N_STATE + n:blk * N_STATE + n + 1])
                    dBx = ph2.tile([128, L], BF16, tag="dBx", name="dBx")
                    nc.vector.tensor_tensor(dBx[:], dtx_t[blk][:], Bt[:],
                                            OP.mult)
                    h = ph2.tile([128, L], BF16, tag="h", name="h")
                    nc.vector.tensor_tensor_scan(
                        h[:], dA[:], dBx[:], 0.0, OP.mult, OP.add)
                    if n == 0:
                        nc.vector.tensor_tensor(y_t[blk][:], h[:], Ct[:],
                                                OP.mult)
                    else:
                        p = ph2.tile([128, L], BF16, tag="p", name="p")
                        nc.vector.tensor_tensor(p[:], h[:], Ct[:], OP.mult)
                        nc.gpsimd.tensor_tensor(y_t[blk][:], y_t[blk][:], p[:],
                                                OP.add)

        # ---------------- phase 3: gate + out-proj ----------------
        with tc.tile_pool(name="ph3", bufs=1) as ph3, \
             tc.tile_pool(name="ph3b", bufs=2) as ph3b:
            for blk in range(NBLK_H):
                # z half, silu, gate
                gz = ph3.tile([128, L], BF16, tag=f"gz{blk}", name=f"gz{blk}")
                for nch in range(NCH):
                    ps = pio.tile([128, 512], F32, tag="pio", name="ps_z")
                    for kb in range(4):
                        nc.tensor.matmul(
                            ps[:],
                            lhsT=w_in_v[:, kb, 1024 + blk * 128:1024 + (blk + 1) * 128],
                            rhs=xT_v[:, kb, 3 + nch * 512:3 + (nch + 1) * 512],
                            start=(kb == 0), stop=(kb == 3),
                        )
                    nc.scalar.activation(gz[:, nch * 512:(nch + 1) * 512],
                                         ps[:], AF.Silu)
                # y = (y + D*xc) * gz
                nc.vector.scalar_tensor_tensor(
                    y_t[blk][:], xc_t[blk][:], D_sb[:, blk:blk + 1],
                    y_t[blk][:], OP.mult, OP.add)
                nc.vector.tensor_tensor(y_t[blk][:], y_t[blk][:], gz[:],
                                        OP.mult)
            for m in range(4):
                for nch in range(NCH):
                    ps = pio.tile([128, 512], F32, tag="pio", name="ps_out")
                    for kb in range(NBLK_H):
                        nc.tensor.matmul(
                            ps[:], lhsT=w_out_v[:, kb, m * 128:(m + 1) * 128],
                            rhs=y_t[kb][:, nch * 512:(nch + 1) * 512],
                            start=(kb == 0), stop=(kb == NBLK_H - 1))
                    ob = ph3b.tile([128, 512], F32, tag="outb", name="outb")
                    nc.scalar.copy(ob[:], ps[:])
                    nc.sync.dma_start(
                        out_d[m * 128:(m + 1) * 128,
                              nch * 512:(nch + 1) * 512], ob[:])

    _split_excess_waits(nc)
    return nc


def _split_excess_waits(nc, max_waits=1):
    """The walrus build rejects instructions carrying more than one
    sync-wait command ("Too many sync wait commands" on Tile's kernel-tail
    Drain, which waits on every loose semaphore). Move excess waits onto
    NoOps placed just before the offender on the same engine."""
    for fn in nc.m.functions:
        for blk in fn.blocks:
            out, changed = [], False
            for inst in blk.instructions:
                si = inst.sync_info
                waits = list(si.on_wait) if si is not None and si.on_wait else []
                if len(waits) > max_waits:
                    extra, keep = waits[:-max_waits], waits[-max_waits:]
                    chunks = [extra[i:i + max_waits]
                              for i in range(0, len(extra), max_waits)]
                    for j, ch in enumerate(chunks):
                        nop = mybir.InstNoOp(
                            name=f"{inst.name}-waitsplit{j}", ins=[], outs=[])
                        nop.engine = inst.engine
                        nop.sync_info = mybir.SyncInfo(on_wait=ch, on_update=[])
                        out.append(nop)
                    si.on_wait = keep
                    changed = True
                out.append(inst)
            if changed:
                blk.instructions = out


_PROG = None


def _get_program():
    global _PROG
    if _PROG is None:
        _PROG = _build_program()
    return _PROG


def _to_pblocks(a, nblk, dtype):
    """[nblk*128, f] -> [128, nblk*f] with [p, blk*f+j] = a[blk*128+p, j]."""
    a = np.ascontiguousarray(a)
    f = a.shape[1] if a.ndim > 1 else 1
    a = a.reshape(nblk, 128, f).transpose(1, 0, 2).reshape(128, nblk * f)
    return np.ascontiguousarray(a.astype(dtype))


def _core_inputs(hs, params, fuse_w, b, dr, h):
    p = params[dr]
    x = hs[b]
    if dr == 1:
        x = x[::-1]
    xTp = np.concatenate(
        [np.zeros((D_MODEL, 3), np.float32), np.ascontiguousarray(x.T)], axis=1)
    xT = _to_pblocks(xTp, 4, BF16NP)  # [128, 4*(L+3)] bf16

    sl_own = slice(h * DH, (h + 1) * DH)
    perm = np.r_[h * DH:(h + 1) * DH, (1 - h) * DH:(2 - h) * DH]

    in_w = p["in_w"]
    w_in_cols = np.concatenate(
        [in_w[:, :D_INNER][:, perm], in_w[:, D_INNER:][:, sl_own]], axis=1)
    w_in = _to_pblocks(w_in_cols, 4, BF16NP)

    conv_w = _to_pblocks(p["conv_w"][perm], NBLK_F, np.float32)
    conv_b = _to_pblocks(p["conv_b"][perm][:, None], NBLK_F, np.float32)
    xproj_w = _to_pblocks(p["xproj_w"][perm], NBLK_F, BF16NP)
    dt_w = np.ascontiguousarray(p["dt_w"][:, sl_own].astype(BF16NP))
    dt_b = _to_pblocks(p["dt_b"][sl_own][:, None], NBLK_H, np.float32)
    A = _to_pblocks(-np.exp(p["A_log"][sl_own]), NBLK_H, np.float32)
    D = _to_pblocks(p["D_skip"][sl_own][:, None], NBLK_H, np.float32)

    fuse_half = fuse_w[:D_MODEL] if dr == 0 else fuse_w[D_MODEL:]
    w_out_full = p["out_w"].astype(np.float64) @ fuse_half.astype(np.float64)
    w_out = _to_pblocks(w_out_full[sl_own].astype(np.float32), 4, BF16NP)

    bcsel = np.zeros((N_STATE, N_STATE * 128), BF16NP)
    for n in range(N_STATE):
        bcsel[n, n * 128:(n + 1) * 128] = 1.0

    return {
        "xT": xT, "w_in": w_in, "conv_w": conv_w, "conv_b": conv_b,
        "xproj_w": xproj_w, "dt_w": dt_w, "dt_b": dt_b, "A": A, "D": D,
        "w_out": w_out, "bcsel": bcsel,
    }


def kernel(_spmd_kwargs=None, **inputs):
    hs = np.asarray(inputs["hidden_states"], dtype=np.float32)
    fuse_w = np.asarray(inputs["fuse_w"], dtype=np.float32)
    fuse_b = np.asarray(inputs["fuse_b"], dtype=np.float32)
    params = []
    for pre in ("fwd_", "bwd_"):
        params.append({k[len(pre):]: np.asarray(v, dtype=np.float32)
                       for k, v in inputs.items() if k.startswith(pre)})

    nc = _get_program()

    in_maps = []
    core_cfg = []
    prep_cache = {}
    for c in range(8):
        b, dr, h = c >> 2, (c >> 1) & 1, c & 1
        core_cfg.append((b, dr, h))
        key = (b, dr, h)
        if key not in prep_cache:
            prep_cache[key] = _core_inputs(hs, params, fuse_w, b, dr, h)
        in_maps.append(prep_cache[key])

    res = run_bass_kernel_spmd(nc, in_maps, core_ids=list(range(8)),
                               **(_spmd_kwargs or {}))

    out = np.zeros((B, L, D_MODEL), dtype=np.float32)
    for c in range(8):
        b, dr, h = core_cfg[c]
        contrib = res.results[c]["out_part"].T  # (L, D_MODEL)
        if dr == 1:
            contrib = contrib[::-1]
        out[b] += contrib
    out += fuse_b[None, None, :]
    if _spmd_kwargs is not None:
        kernel._last_result = res
    return out


# revision 22
# speedup vs baseline: 1.1792x; 1.1792x over previous
"""BiMambaBlock on 8 Trainium2 NeuronCores.

Sharding: core c = (batch b, direction d, d_inner-half h) with
b = c>>2, d = (c>>1)&1, h = c&1.  Every core runs the same program on
different data (weights sliced/permuted per core on the host):

  - host feeds x[b].T in bf16 (flipped along L for bwd cores, padded
    with 3 leading zero cols for the causal conv), so the device always
    runs a *forward* mamba mixer in channels-on-partitions layout [d, L].
  - each core computes the full xc = silu(conv(x @ in_w_xi)) over all
    1024 channels (so the xproj contraction over d_inner stays local,
    no collectives), but scans only its 512-channel half (the host
    permutes weights so the own half is always channel blocks 0-3).
  - out_proj and the final fuse matmul are folded on the host into one
    [512ch, 512dm] weight; each core emits a partial [512dm, L] f32
    which the host transposes/flips/sums.

Everything except PSUM accumulation runs in bf16 (the scan keeps fp32
internal state).  The full L=2048 is processed unchunked: the
tensor_tensor_scan has a ~2.5us fixed cost, so one long scan per
(block, n) beats four chained chunk scans.  B_n / C_n rows are
broadcast across partitions with PE selector matmuls (lhsT one-hot
columns), drained to bf16 so the dBx / C-mul tensor_tensor ops hit the
DVE 2x bf16 mode.  The y += h*C accumulation chain runs on the
otherwise-idle GPSIMD engine.
"""
import os
import sys

for _p in ("/opt/trn_rl_repo",):
    if os.path.isdir(_p) and _p not in sys.path:
        sys.path.insert(0, _p)

from contextlib import ExitStack

import ml_dtypes
import numpy as np

from concourse import bass, mybir, tile
from concourse.bass_utils import run_bass_kernel_spmd

F32 = mybir.dt.float32
BF16 = mybir.dt.bfloat16
AF = mybir.ActivationFunctionType
OP = mybir.AluOpType

D_MODEL = 512
D_INNER = 1024
DH = 512
N_STATE = 16
D_CONV = 4
DT_RANK = 32
B = 2
L = 2048
LP = L + 3

NBLK_F = D_INNER // 128  # 8 channel blocks for conv/xproj
NBLK_H = DH // 128       # 4 scan blocks

NCH = L // 512           # 512-wide matmul N-chunks

BF16NP = ml_dtypes.bfloat16


def _build_program():
    nc = bass.Bass(trn_type="TRN2", target_bir_lowering=False, debug=False)

    xT_d = nc.dram_tensor("xT", [128, 4 * LP], BF16, kind="ExternalInput")
    w_in_d = nc.dram_tensor("w_in", [128, 4 * 1536], BF16, kind="ExternalInput")
    conv_w_d = nc.dram_tensor("conv_w", [128, NBLK_F * D_CONV], F32, kind="ExternalInput")
    conv_b_d = nc.dram_tensor("conv_b", [128, NBLK_F], F32, kind="ExternalInput")
    xproj_w_d = nc.dram_tensor("xproj_w", [128, NBLK_F * 64], BF16, kind="ExternalInput")
    dt_w_d = nc.dram_tensor("dt_w", [DT_RANK, DH], BF16, kind="ExternalInput")
    dt_b_d = nc.dram_tensor("dt_b", [128, NBLK_H], F32, kind="ExternalInput")
    A_d = nc.dram_tensor("A", [128, NBLK_H * N_STATE], F32, kind="ExternalInput")
    D_d = nc.dram_tensor("D", [128, NBLK_H], F32, kind="ExternalInput")
    w_out_d = nc.dram_tensor("w_out", [128, 4 * D_MODEL], BF16, kind="ExternalInput")
    bcsel_d = nc.dram_tensor("bcsel", [N_STATE, N_STATE * 128], BF16,
                             kind="ExternalInput")
    out_d = nc.dram_tensor("out_part", [D_MODEL, L], F32, kind="ExternalOutput")

    with tile.TileContext(nc) as tc, ExitStack() as ctx:
        # ---------------- global pools / persistent tiles ----------------
        wp = ctx.enter_context(tc.tile_pool(name="weights", bufs=1))

        xT = wp.tile([128, 4 * LP], BF16, tag="xT")
        w_in = wp.tile([128, 4 * 1536], BF16, tag="w_in")
        conv_w = wp.tile([128, NBLK_F * D_CONV], F32, tag="conv_w")
        conv_b = wp.tile([128, NBLK_F], F32, tag="conv_b")
        xproj_w = wp.tile([128, NBLK_F * 64], BF16, tag="xproj_w")
        dt_w = wp.tile([DT_RANK, DH], BF16, tag="dt_w")
        dt_b = wp.tile([128, NBLK_H], F32, tag="dt_b")
        A_sb = wp.tile([128, NBLK_H * N_STATE], F32, tag="A")
        D_sb = wp.tile([128, NBLK_H], F32, tag="D")
        w_out = wp.tile([128, 4 * D_MODEL], BF16, tag="w_out")
        bcsel = wp.tile([N_STATE, N_STATE * 128], BF16, tag="bcsel")

        for t, d in [(xT, xT_d), (w_in, w_in_d), (conv_w, conv_w_d),
                     (conv_b, conv_b_d), (xproj_w, xproj_w_d), (dt_w, dt_w_d),
                     (dt_b, dt_b_d), (A_sb, A_d), (D_sb, D_d), (w_out, w_out_d),
                     (bcsel, bcsel_d)]:
            nc.sync.dma_start(t[:], d[:])

        xT_v = xT[:].rearrange("p (k l) -> p k l", k=4)
        w_in_v = w_in[:].rearrange("p (k m) -> p k m", k=4)
        xproj_v = xproj_w[:].rearrange("p (k f) -> p k f", k=NBLK_F)
        w_out_v = w_out[:].rearrange("p (k m) -> p k m", k=4)

        pio = ctx.enter_context(tc.tile_pool(name="pio", bufs=4, space="PSUM"))
        pdbc = ctx.enter_context(tc.tile_pool(name="pdbc", bufs=2, space="PSUM"))

        glob = ctx.enter_context(tc.tile_pool(name="glob", bufs=1))
        xc_t = [glob.tile([128, L], BF16, tag=f"xc{i}", name=f"xc{i}")
                for i in range(NBLK_H)]  # own-half xc, live till the end
        dt_t = [glob.tile([128, L], BF16, tag=f"dt{i}", name=f"dt{i}")
                for i in range(NBLK_H)]
        dtx_t = [glob.tile([128, L], BF16, tag=f"dtx{i}", name=f"dtx{i}")
                 for i in range(NBLK_H)]
        y_t = [glob.tile([128, L], BF16, tag=f"y{i}", name=f"y{i}")
               for i in range(NBLK_H)]
        dbc = glob.tile([64, L], BF16, tag="dbc")
        B_sb = glob.tile([N_STATE, L], BF16, tag="Brows")
        C_sb = glob.tile([N_STATE, L], BF16, tag="Crows")

        def in_proj_block(m0, xi, xi_off, n_cols):
            """matmul w_in cols [m0, m0+128) x xT -> xi[:, xi_off:...]"""
            for nch in range(0, n_cols, 512):
                w = min(512, n_cols - nch)
                ps = pio.tile([128, 512], F32, tag="pio", name="ps_in")
                for kb in range(4):
                    nc.tensor.matmul(
                        ps[:, 0:w],
                        lhsT=w_in_v[:, kb, m0:m0 + 128],
                        rhs=xT_v[:, kb, nch:nch + w],
                        start=(kb == 0), stop=(kb == 3),
                    )
                nc.scalar.copy(xi[:, xi_off + nch:xi_off + nch + w], ps[:, 0:w])

        # ---------------- phase 1: xc / xproj / dt ----------------
        with tc.tile_pool(name="ph1", bufs=1) as ph1, \
             tc.tile_pool(name="ph1b", bufs=2) as ph1b:
            for blk in range(NBLK_F):
                xi = ph1b.tile([128, LP], BF16, tag="xi", name="xi")
                in_proj_block(blk * 128, xi, 0, LP)
                # conv taps on ACT (per-partition scale), pair-adds on DVE
                tk = []
                for k in range(D_CONV):
                    t = ph1b.tile([128, L], BF16, tag=f"ct{k}", name=f"ct{k}")
                    nc.scalar.mul(t[:], xi[:, k:k + L],
                                  conv_w[:, blk * 4 + k:blk * 4 + k + 1])
                    tk.append(t)
                nc.vector.tensor_tensor(tk[0][:], tk[0][:], tk[1][:], OP.add)
                nc.vector.tensor_tensor(tk[2][:], tk[2][:], tk[3][:], OP.add)
                acc = tk[0]
                nc.vector.tensor_tensor(acc[:], acc[:], tk[2][:], OP.add)
                if blk < NBLK_H:
                    xc = xc_t[blk]
                else:
                    xc = ph1.tile([128, L], BF16, tag=f"xcO{blk}",
                                  name=f"xcO{blk}")
                nc.scalar.activation(xc[:], acc[:], AF.Silu,
                                     bias=conv_b[:, blk:blk + 1])
                if blk < NBLK_H:
                    xc_t[blk] = xc
                else:
                    xc_t.append(xc)

            # xproj -> dbc.T [64, L]
            for nch in range(NCH):
                ps = pdbc.tile([64, 512], F32, tag="pdbc", name="ps_dbc")
                for kb in range(NBLK_F):
                    nc.tensor.matmul(
                        ps[:], lhsT=xproj_v[:, kb, :],
                        rhs=xc_t[kb][:, nch * 512:(nch + 1) * 512],
                        start=(kb == 0), stop=(kb == NBLK_F - 1),
                    )
                nc.scalar.copy(dbc[:, nch * 512:(nch + 1) * 512], ps[:])

            # dt = softplus(dt_raw.T + dt_b) = ln(1 + exp(.))
            for m in range(NBLK_H):
                dte = ph1b.tile([128, L], BF16, tag="dte", name="dte")
                for nch in range(NCH):
                    ps = pio.tile([128, 512], F32, tag="pio", name="ps_dt")
                    nc.tensor.matmul(
                        ps[:], lhsT=dt_w[:, m * 128:(m + 1) * 128],
                        rhs=dbc[0:DT_RANK, nch * 512:(nch + 1) * 512],
                        start=True, stop=True)
                    nc.scalar.activation(dte[:, nch * 512:(nch + 1) * 512],
                                         ps[:], AF.Exp, bias=dt_b[:, m:m + 1])
                nc.scalar.activation(dt_t[m][:], dte[:], AF.Ln, bias=1.0)
                nc.vector.tensor_tensor(dtx_t[m][:], dt_t[m][:], xc_t[m][:],
                                        OP.mult)

            nc.sync.dma_start(B_sb[:], dbc[32:48, :])
            nc.sync.dma_start(C_sb[:], dbc[48:64, :])

        # ---------------- phase 2: scan over (n, blk) ----------------
        with tc.tile_pool(name="ph2", bufs=2) as ph2:
            for n in range(N_STATE):
                Bt = ph2.tile([128, L], BF16, tag="Bt", name="Bt")
                Ct = ph2.tile([128, L], BF16, tag="Ct", name="Ct")
                for src, dst in ((B_sb, Bt), (C_sb, Ct)):
                    for nch in range(NCH):
                        ps = pio.tile([128, 512], F32, tag="pio", name="ps_bc")
                        nc.tensor.matmul(
                            ps[:], lhsT=bcsel[:, n * 128:(n + 1) * 128],
                            rhs=src[:, nch * 512:(nch + 1) * 512],
                            start=True, stop=True)
                        nc.scalar.copy(dst[:, nch * 512:(nch + 1) * 512], ps[:])
                for blk in range(NBLK_H):
                    dA = ph2.tile([128, L], BF16, tag="dA", name="dA")
                    nc.scalar.activation(
                        dA[:], dt_t[blk][:], AF.Exp,
                        scale=A_sb[:, blk * N_STATE + n:blk * N_STATE + n + 1])
                    dBx = ph2.tile([128, L], BF16, tag="dBx", name="dBx")
                    nc.vector.tensor_tensor(dBx[:], dtx_t[blk][:], Bt[:],
                                            OP.mult)
                    h = ph2.tile([128, L], BF16, tag="h", name="h")
                    nc.vector.tensor_tensor_scan(
                        h[:], dA[:], dBx[:], 0.0, OP.mult, OP.add)
                    if n == 0:
                        nc.vector.tensor_tensor(y_t[blk][:], h[:], Ct[:],
                                                OP.mult)
                    else:
                        p = ph2.tile([128, L], BF16, tag="p", name="p")
                        nc.vector.tensor_tensor(p[:], h[:], Ct[:], OP.mult)
                        nc.vector.tensor_tensor(y_t[blk][:], y_t[blk][:], p[:],
                                                OP.add)

        # ---------------- phase 3: gate + out-proj ----------------
        with tc.tile_pool(name="ph3", bufs=1) as ph3, \
             tc.tile_pool(name="ph3b", bufs=2) as ph3b:
            for blk in range(NBLK_H):
                # z half, silu, gate
                gz = ph3.tile([128, L], BF16, tag=f"gz{blk}", name=f"gz{blk}")
                for nch in range(NCH):
                    ps = pio.tile([128, 512], F32, tag="pio", name="ps_z")
                    for kb in range(4):
                        nc.tensor.matmul(
                            ps[:],
                            lhsT=w_in_v[:, kb, 1024 + blk * 128:1024 + (blk + 1) * 128],
                            rhs=xT_v[:, kb, 3 + nch * 512:3 + (nch + 1) * 512],
                            start=(kb == 0), stop=(kb == 3),
                        )
                    nc.scalar.activation(gz[:, nch * 512:(nch + 1) * 512],
                                         ps[:], AF.Silu)
                # y = (y + D*xc) * gz
                nc.vector.scalar_tensor_tensor(
                    y_t[blk][:], xc_t[blk][:], D_sb[:, blk:blk + 1],
                    y_t[blk][:], OP.mult, OP.add)
                nc.vector.tensor_tensor(y_t[blk][:], y_t[blk][:], gz[:],
                                        OP.mult)
            for m in range(4):
                for nch in range(NCH):
                    ps = pio.tile([128, 512], F32, tag="pio", name="ps_out")
                    for kb in range(NBLK_H):
                        nc.tensor.matmul(
                            ps[:], lhsT=w_out_v[:, kb, m * 128:(m + 1) * 128],
                            rhs=y_t[kb][:, nch * 512:(nch + 1) * 512],
                            start=(kb == 0), stop=(kb == NBLK_H - 1))
                    ob = ph3b.tile([128, 512], F32, tag="outb", name="outb")
                    nc.scalar.copy(ob[:], ps[:])
                    nc.sync.dma_start(
                        out_d[m * 128:(m + 1) * 128,
                              nch * 512:(nch + 1) * 512], ob[:])

    _split_excess_waits(nc)
    return nc


def _split_excess_waits(nc, max_waits=1):
    """The walrus build rejects instructions carrying more than one
    sync-wait command ("Too many sync wait commands" on Tile's kernel-tail
    Drain, which waits on every loose semaphore). Move excess waits onto
    NoOps placed just before the offender on the same engine."""
    for fn in nc.m.functions:
        for blk in fn.blocks:
            out, changed = [], False
            for inst in blk.instructions:
                si = inst.sync_info
                waits = list(si.on_wait) if si is not None and si.on_wait else []
                if len(waits) > max_waits:
                    extra, keep = waits[:-max_waits], waits[-max_waits:]
                    chunks = [extra[i:i + max_waits]
                              for i in range(0, len(extra), max_waits)]
                    for j, ch in enumerate(chunks):
                        nop = mybir.InstNoOp(
                            name=f"{inst.name}-waitsplit{j}", ins=[], outs=[])
                        nop.engine = inst.engine
                        nop.sync_info = mybir.SyncInfo(on_wait=ch, on_update=[])
                        out.append(nop)
                    si.on_wait = keep
                    changed = True
                out.append(inst)
            if changed:
                blk.instructions = out


_PROG = None


def _get_program():
    global _PROG
    if _PROG is None:
        _PROG = _build_program()
    return _PROG


def _to_pblocks(a, nblk, dtype):
    """[nblk*128, f] -> [128, nblk*f] with [p, blk*f+j] = a[blk*128+p, j]."""
    a = np.ascontiguousarray(a)
    f = a.shape[1] if a.ndim > 1 else 1
    a = a.reshape(nblk, 128, f).transpose(1, 0, 2).reshape(128, nblk * f)
    return np.ascontiguousarray(a.astype(dtype))


def _core_inputs(hs, params, fuse_w, b, dr, h):
    p = params[dr]
    x = hs[b]
    if dr == 1:
        x = x[::-1]
    xTp = np.concatenate(
        [np.zeros((D_MODEL, 3), np.float32), np.ascontiguousarray(x.T)], axis=1)
    xT = _to_pblocks(xTp, 4, BF16NP)  # [128, 4*(L+3)] bf16

    sl_own = slice(h * DH, (h + 1) * DH)
    perm = np.r_[h * DH:(h + 1) * DH, (1 - h) * DH:(2 - h) * DH]

    in_w = p["in_w"]
    w_in_cols = np.concatenate(
        [in_w[:, :D_INNER][:, perm], in_w[:, D_INNER:][:, sl_own]], axis=1)
    w_in = _to_pblocks(w_in_cols, 4, BF16NP)

    conv_w = _to_pblocks(p["conv_w"][perm], NBLK_F, np.float32)
    conv_b = _to_pblocks(p["conv_b"][perm][:, None], NBLK_F, np.float32)
    xproj_w = _to_pblocks(p["xproj_w"][perm], NBLK_F, BF16NP)
    dt_w = np.ascontiguousarray(p["dt_w"][:, sl_own].astype(BF16NP))
    dt_b = _to_pblocks(p["dt_b"][sl_own][:, None], NBLK_H, np.float32)
    A = _to_pblocks(-np.exp(p["A_log"][sl_own]), NBLK_H, np.float32)
    D = _to_pblocks(p["D_skip"][sl_own][:, None], NBLK_H, np.float32)

    fuse_half = fuse_w[:D_MODEL] if dr == 0 else fuse_w[D_MODEL:]
    w_out_full = p["out_w"].astype(np.float64) @ fuse_half.astype(np.float64)
    w_out = _to_pblocks(w_out_full[sl_own].astype(np.float32), 4, BF16NP)

    bcsel = np.zeros((N_STATE, N_STATE * 128), BF16NP)
    for n in range(N_STATE):
        bcsel[n, n * 128:(n + 1) * 128] = 1.0

    return {
        "xT": xT, "w_in": w_in, "conv_w": conv_w, "conv_b": conv_b,
        "xproj_w": xproj_w, "dt_w": dt_w, "dt_b": dt_b, "A": A, "D": D,
        "w_out": w_out, "bcsel": bcsel,
    }


def kernel(_spmd_kwargs=None, **inputs):
    hs = np.asarray(inputs["hidden_states"], dtype=np.float32)
    fuse_w = np.asarray(inputs["fuse_w"], dtype=np.float32)
    fuse_b = np.asarray(inputs["fuse_b"], dtype=np.float32)
    params = []
    for pre in ("fwd_", "bwd_"):
        params.append({k[len(pre):]: np.asarray(v, dtype=np.float32)
                       for k, v in inputs.items() if k.startswith(pre)})

    nc = _get_program()

    in_maps = []
    core_cfg = []
    prep_cache = {}
    for c in range(8):
        b, dr, h = c >> 2, (c >> 1) & 1, c & 1
        core_cfg.append((b, dr, h))
        key = (b, dr, h)
        if key not in prep_cache:
            prep_cache[key] = _core_inputs(hs, params, fuse_w, b, dr, h)
        in_maps.append(prep_cache[key])

    res = run_bass_kernel_spmd(nc, in_maps, core_ids=list(range(8)),
                               **(_spmd_kwargs or {}))

    out = np.zeros((B, L, D_MODEL), dtype=np.float32)
    for c in range(8):
        b, dr, h = core_cfg[c]
        contrib = res.results[c]["out_part"].T  # (L, D_MODEL)
        if dr == 1:
            contrib = contrib[::-1]
        out[b] += contrib
    out += fuse_b[None, None, :]
    if _spmd_kwargs is not None:
        kernel._last_result = res
    return out


# revision 31
# speedup vs baseline: 1.2611x; 1.0694x over previous
"""BiMambaBlock on 8 Trainium2 NeuronCores.

Sharding: core c = (batch b, direction d, d_inner-half h) with
b = c>>2, d = (c>>1)&1, h = c&1.  Every core runs the same program on
different data (weights sliced/permuted per core on the host):

  - host feeds x[b].T in bf16 (flipped along L for bwd cores, padded
    with 3 leading zero cols for the causal conv), so the device always
    runs a *forward* mamba mixer in channels-on-partitions layout [d, L].
  - each core computes the full xc = silu(conv(x @ in_w_xi)) over all
    1024 channels (so the xproj contraction over d_inner stays local,
    no collectives), but scans only its 512-channel half (the host
    permutes weights so the own half is always channel blocks 0-3).
  - out_proj and the final fuse matmul are folded on the host into one
    [512ch, 512dm] weight; each core emits a partial [512dm, L] f32
    which the host transposes/flips/sums.

Everything except PSUM accumulation runs in bf16 (the scan keeps fp32
internal state).  The full L=2048 is processed unchunked: the
tensor_tensor_scan has a ~2.5us fixed cost, so one long scan per
(block, n) beats four chained chunk scans.  B_n / C_n rows are
broadcast across partitions with PE selector matmuls (lhsT one-hot
columns), drained to bf16 so the dBx / C-mul tensor_tensor ops hit the
DVE 2x bf16 mode.  The y += h*C accumulation chain runs on the
otherwise-idle GPSIMD engine.
"""
import os
import sys

for _p in ("/opt/trn_rl_repo",):
    if os.path.isdir(_p) and _p not in sys.path:
        sys.path.insert(0, _p)

from contextlib import ExitStack

import ml_dtypes
import numpy as np

from concourse import bass, mybir, tile
from concourse.bass_utils import run_bass_kernel_spmd

F32 = mybir.dt.float32
BF16 = mybir.dt.bfloat16
AF = mybir.ActivationFunctionType
OP = mybir.AluOpType

D_MODEL = 512
D_INNER = 1024
DH = 512
N_STATE = 16
D_CONV = 4
DT_RANK = 32
B = 2
L = 2048
LP = L + 3

NBLK_F = D_INNER // 128  # 8 channel blocks for conv/xproj
NBLK_H = DH // 128       # 4 scan blocks

NCH = L // 512           # 512-wide matmul N-chunks

BF16NP = ml_dtypes.bfloat16


def _build_program():
    nc = bass.Bass(trn_type="TRN2", target_bir_lowering=False, debug=False)

    xT_d = nc.dram_tensor("xT", [128, 4 * LP], BF16, kind="ExternalInput")
    w_in_d = nc.dram_tensor("w_in", [128, 4 * 1536], BF16, kind="ExternalInput")
    conv_w_d = nc.dram_tensor("conv_w", [128, NBLK_F * D_CONV], F32, kind="ExternalInput")
    conv_b_d = nc.dram_tensor("conv_b", [128, NBLK_F], F32, kind="ExternalInput")
    xproj_w_d = nc.dram_tensor("xproj_w", [128, NBLK_F * 64], BF16, kind="ExternalInput")
    dt_w_d = nc.dram_tensor("dt_w", [DT_RANK, DH], BF16, kind="ExternalInput")
    dt_b_d = nc.dram_tensor("dt_b", [128, NBLK_H], F32, kind="ExternalInput")
    A_d = nc.dram_tensor("A", [128, NBLK_H * N_STATE], F32, kind="ExternalInput")
    D_d = nc.dram_tensor("D", [128, NBLK_H], F32, kind="ExternalInput")
    w_out_d = nc.dram_tensor("w_out", [128, 4 * D_MODEL], BF16, kind="ExternalInput")
    bcsel_d = nc.dram_tensor("bcsel", [N_STATE, N_STATE * 128], BF16,
                             kind="ExternalInput")
    out_d = nc.dram_tensor("out_part", [D_MODEL, L], F32, kind="ExternalOutput")

    with tile.TileContext(nc) as tc, ExitStack() as ctx:
        # ---------------- global pools / persistent tiles ----------------
        wp = ctx.enter_context(tc.tile_pool(name="weights", bufs=1))

        xT = wp.tile([128, 4 * LP], BF16, tag="xT")
        w_in = wp.tile([128, 4 * 1536], BF16, tag="w_in")
        conv_w = wp.tile([128, NBLK_F * D_CONV], F32, tag="conv_w")
        conv_b = wp.tile([128, NBLK_F], F32, tag="conv_b")
        xproj_w = wp.tile([128, NBLK_F * 64], BF16, tag="xproj_w")
        dt_w = wp.tile([DT_RANK, DH], BF16, tag="dt_w")
        dt_b = wp.tile([128, NBLK_H], F32, tag="dt_b")
        A_sb = wp.tile([128, NBLK_H * N_STATE], F32, tag="A")
        D_sb = wp.tile([128, NBLK_H], F32, tag="D")
        w_out = wp.tile([128, 4 * D_MODEL], BF16, tag="w_out")
        bcsel = wp.tile([N_STATE, N_STATE * 128], BF16, tag="bcsel")

        for t, d in [(xT, xT_d), (w_in, w_in_d), (conv_w, conv_w_d),
                     (conv_b, conv_b_d), (xproj_w, xproj_w_d), (dt_w, dt_w_d),
                     (dt_b, dt_b_d), (A_sb, A_d), (D_sb, D_d), (w_out, w_out_d),
                     (bcsel, bcsel_d)]:
            nc.sync.dma_start(t[:], d[:])

        xT_v = xT[:].rearrange("p (k l) -> p k l", k=4)
        w_in_v = w_in[:].rearrange("p (k m) -> p k m", k=4)
        xproj_v = xproj_w[:].rearrange("p (k f) -> p k f", k=NBLK_F)
        w_out_v = w_out[:].rearrange("p (k m) -> p k m", k=4)

        pio = ctx.enter_context(tc.tile_pool(name="pio", bufs=4, space="PSUM"))
        pdbc = ctx.enter_context(tc.tile_pool(name="pdbc", bufs=2, space="PSUM"))

        glob = ctx.enter_context(tc.tile_pool(name="glob", bufs=1))
        xc_t = [glob.tile([128, L], BF16, tag=f"xc{i}", name=f"xc{i}")
                for i in range(NBLK_H)]  # own-half xc, live till the end
        dt_t = [glob.tile([128, L], BF16, tag=f"dt{i}", name=f"dt{i}")
                for i in range(NBLK_H)]
        dtx_t = [glob.tile([128, L], BF16, tag=f"dtx{i}", name=f"dtx{i}")
                 for i in range(NBLK_H)]
        y_t = [glob.tile([128, L], BF16, tag=f"y{i}", name=f"y{i}")
               for i in range(NBLK_H)]
        dbc = glob.tile([64, L], BF16, tag="dbc")
        B_sb = glob.tile([N_STATE, L], BF16, tag="Brows")
        C_sb = glob.tile([N_STATE, L], BF16, tag="Crows")

        def in_proj_block(m0, xi, xi_off, n_cols):
            """matmul w_in cols [m0, m0+128) x xT -> xi[:, xi_off:...]"""
            for gi, nch in enumerate(range(0, n_cols, 512)):
                w = min(512, n_cols - nch)
                ps = pio.tile([128, 512], F32, tag="pio", name="ps_in")
                for kb in range(4):
                    nc.tensor.matmul(
                        ps[:, 0:w],
                        lhsT=w_in_v[:, kb, m0:m0 + 128],
                        rhs=xT_v[:, kb, nch:nch + w],
                        start=(kb == 0), stop=(kb == 3),
                    )
                dst = xi[:, xi_off + nch:xi_off + nch + w]
                # balance phase-1: alternate PSUM drains between ACT and DVE
                if gi % 2 == 0:
                    nc.scalar.copy(dst, ps[:, 0:w])
                else:
                    nc.vector.tensor_copy(dst, ps[:, 0:w])

        # ---------------- phase 1: xc / xproj / dt ----------------
        with tc.tile_pool(name="ph1", bufs=1) as ph1, \
             tc.tile_pool(name="ph1b", bufs=2) as ph1b:
            for blk in range(NBLK_F):
                xi = ph1b.tile([128, LP], BF16, tag="xi", name="xi")
                in_proj_block(blk * 128, xi, 0, LP)
                # conv taps: even offsets (4B-aligned, DVE 4x tensor_scalar),
                # odd offsets on ACT; pair-adds on DVE
                tk = []
                for k in range(D_CONV):
                    t = ph1b.tile([128, L], BF16, tag=f"ct{k}", name=f"ct{k}")
                    w_col = conv_w[:, blk * 4 + k:blk * 4 + k + 1]
                    if k % 2 == 0:
                        nc.vector.tensor_scalar_mul(t[:], xi[:, k:k + L], w_col)
                    else:
                        nc.scalar.mul(t[:], xi[:, k:k + L], w_col)
                    tk.append(t)
                nc.vector.tensor_tensor(tk[0][:], tk[0][:], tk[1][:], OP.add)
                nc.vector.tensor_tensor(tk[2][:], tk[2][:], tk[3][:], OP.add)
                acc = tk[0]
                nc.vector.tensor_tensor(acc[:], acc[:], tk[2][:], OP.add)
                if blk < NBLK_H:
                    xc = xc_t[blk]
                else:
                    xc = ph1.tile([128, L], BF16, tag=f"xcO{blk}",
                                  name=f"xcO{blk}")
                nc.scalar.activation(xc[:], acc[:], AF.Silu,
                                     bias=conv_b[:, blk:blk + 1])
                if blk < NBLK_H:
                    xc_t[blk] = xc
                else:
                    xc_t.append(xc)

            # xproj -> dbc.T [64, L]
            for nch in range(NCH):
                ps = pdbc.tile([64, 512], F32, tag="pdbc", name="ps_dbc")
                for kb in range(NBLK_F):
                    nc.tensor.matmul(
                        ps[:], lhsT=xproj_v[:, kb, :],
                        rhs=xc_t[kb][:, nch * 512:(nch + 1) * 512],
                        start=(kb == 0), stop=(kb == NBLK_F - 1),
                    )
                nc.scalar.copy(dbc[:, nch * 512:(nch + 1) * 512], ps[:])

            # dt = softplus(dt_raw.T + dt_b) = ln(1 + exp(.))
            for m in range(NBLK_H):
                dte = ph1b.tile([128, L], BF16, tag="dte", name="dte")
                for nch in range(NCH):
                    ps = pio.tile([128, 512], F32, tag="pio", name="ps_dt")
                    nc.tensor.matmul(
                        ps[:], lhsT=dt_w[:, m * 128:(m + 1) * 128],
                        rhs=dbc[0:DT_RANK, nch * 512:(nch + 1) * 512],
                        start=True, stop=True)
                    nc.scalar.activation(dte[:, nch * 512:(nch + 1) * 512],
                                         ps[:], AF.Exp, bias=dt_b[:, m:m + 1])
                nc.scalar.activation(dt_t[m][:], dte[:], AF.Ln, bias=1.0)
                nc.vector.tensor_tensor(dtx_t[m][:], dt_t[m][:], xc_t[m][:],
                                        OP.mult)

            nc.sync.dma_start(B_sb[:], dbc[32:48, :])
            nc.sync.dma_start(C_sb[:], dbc[48:64, :])

        # ---------------- phase 2: scan over (n, blk) ----------------
        with tc.tile_pool(name="ph2", bufs=2) as ph2:
            for n in range(N_STATE):
                Bt = ph2.tile([128, L], BF16, tag="Bt", name="Bt")
                Ct = ph2.tile([128, L], BF16, tag="Ct", name="Ct")
                for src_r, dst in ((B_sb, Bt), (C_sb, Ct)):
                    for nch in range(NCH):
                        ps = pio.tile([128, 512], F32, tag="pio", name="ps_bc")
                        nc.tensor.matmul(
                            ps[:], lhsT=bcsel[:, n * 128:(n + 1) * 128],
                            rhs=src_r[:, nch * 512:(nch + 1) * 512],
                            start=True, stop=True)
                        nc.scalar.copy(dst[:, nch * 512:(nch + 1) * 512], ps[:])
                for blk in range(NBLK_H):
                    dA = ph2.tile([128, L], BF16, tag="dA", name="dA")
                    nc.scalar.activation(
                        dA[:], dt_t[blk][:], AF.Exp,
                        scale=A_sb[:, blk * N_STATE + n:blk * N_STATE + n + 1])
                    dBx = ph2.tile([128, L], BF16, tag="dBx", name="dBx")
                    nc.vector.tensor_tensor(dBx[:], dtx_t[blk][:], Bt[:],
                                            OP.mult)
                    h = ph2.tile([128, L], BF16, tag="h", name="h")
                    nc.vector.tensor_tensor_scan(
                        h[:], dA[:], dBx[:], 0.0, OP.mult, OP.add)
                    if n == 0:
                        nc.vector.tensor_tensor(y_t[blk][:], h[:], Ct[:],
                                                OP.mult)
                    else:
                        p = ph2.tile([128, L], BF16, tag="p", name="p")
                        nc.vector.tensor_tensor(p[:], h[:], Ct[:], OP.mult)
                        nc.vector.tensor_tensor(y_t[blk][:], y_t[blk][:], p[:],
                                                OP.add)

        # ---------------- phase 3: gate + out-proj ----------------
        with tc.tile_pool(name="ph3", bufs=1) as ph3, \
             tc.tile_pool(name="ph3b", bufs=2) as ph3b:
            for blk in range(NBLK_H):
                # z half, silu, gate
                gz = ph3.tile([128, L], BF16, tag=f"gz{blk}", name=f"gz{blk}")
                for nch in range(NCH):
                    ps = pio.tile([128, 512], F32, tag="pio", name="ps_z")
                    for kb in range(4):
                        nc.tensor.matmul(
                            ps[:],
                            lhsT=w_in_v[:, kb, 1024 + blk * 128:1024 + (blk + 1) * 128],
                            rhs=xT_v[:, kb, 3 + nch * 512:3 + (nch + 1) * 512],
                            start=(kb == 0), stop=(kb == 3),
                        )
                    nc.scalar.activation(gz[:, nch * 512:(nch + 1) * 512],
                                         ps[:], AF.Silu)
                # y = (y + D*xc) * gz
                nc.vector.scalar_tensor_tensor(
                    y_t[blk][:], xc_t[blk][:], D_sb[:, blk:blk + 1],
                    y_t[blk][:], OP.mult, OP.add)
                nc.vector.tensor_tensor(y_t[blk][:], y_t[blk][:], gz[:],
                                        OP.mult)
            for m in range(4):
                for nch in range(NCH):
                    ps = pio.tile([128, 512], F32, tag="pio", name="ps_out")
                    for kb in range(NBLK_H):
                        nc.tensor.matmul(
                            ps[:], lhsT=w_out_v[:, kb, m * 128:(m + 1) * 128],
                            rhs=y_t[kb][:, nch * 512:(nch + 1) * 512],
                            start=(kb == 0), stop=(kb == NBLK_H - 1))
                    ob = ph3b.tile([128, 512], F32, tag="outb", name="outb")
                    nc.scalar.copy(ob[:], ps[:])
                    nc.sync.dma_start(
                        out_d[m * 128:(m + 1) * 128,
                              nch * 512:(nch + 1) * 512], ob[:])

    _split_excess_waits(nc)
    return nc


def _split_excess_waits(nc, max_waits=1):
    """The walrus build rejects instructions carrying more than one
    sync-wait command ("Too many sync wait commands" on Tile's kernel-tail
    Drain, which waits on every loose semaphore). Move excess waits onto
    NoOps placed just before the offender on the same engine."""
    for fn in nc.m.functions:
        for blk in fn.blocks:
            out, changed = [], False
            for inst in blk.instructions:
                si = inst.sync_info
                waits = list(si.on_wait) if si is not None and si.on_wait else []
                if len(waits) > max_waits:
                    extra, keep = waits[:-max_waits], waits[-max_waits:]
                    chunks = [extra[i:i + max_waits]
                              for i in range(0, len(extra), max_waits)]
                    for j, ch in enumerate(chunks):
                        nop = mybir.InstNoOp(
                            name=f"{inst.name}-waitsplit{j}", ins=[], outs=[])
                        nop.engine = inst.engine
                        nop.sync_info = mybir.SyncInfo(on_wait=ch, on_update=[])
                        out.append(nop)
                    si.on_wait = keep
                    changed = True
                out.append(inst)
            if changed:
                blk.instructions = out


_PROG = None


def _get_program():
    global _PROG
    if _PROG is None:
        _PROG = _build_program()
    return _PROG


def _to_pblocks(a, nblk, dtype):
    """[nblk*128, f] -> [128, nblk*f] with [p, blk*f+j] = a[blk*128+p, j]."""
    a = np.ascontiguousarray(a)
    f = a.shape[1] if a.ndim > 1 else 1
    a = a.reshape(nblk, 128, f).transpose(1, 0, 2).reshape(128, nblk * f)
    return np.ascontiguousarray(a.astype(dtype))


def _core_inputs(hs, params, fuse_w, b, dr, h):
    p = params[dr]
    x = hs[b]
    if dr == 1:
        x = x[::-1]
    xTp = np.concatenate(
        [np.zeros((D_MODEL, 3), np.float32), np.ascontiguousarray(x.T)], axis=1)
    xT = _to_pblocks(xTp, 4, BF16NP)  # [128, 4*(L+3)] bf16

    sl_own = slice(h * DH, (h + 1) * DH)
    perm = np.r_[h * DH:(h + 1) * DH, (1 - h) * DH:(2 - h) * DH]

    in_w = p["in_w"]
    w_in_cols = np.concatenate(
        [in_w[:, :D_INNER][:, perm], in_w[:, D_INNER:][:, sl_own]], axis=1)
    w_in = _to_pblocks(w_in_cols, 4, BF16NP)

    conv_w = _to_pblocks(p["conv_w"][perm], NBLK_F, np.float32)
    conv_b = _to_pblocks(p["conv_b"][perm][:, None], NBLK_F, np.float32)
    xproj_w = _to_pblocks(p["xproj_w"][perm], NBLK_F, BF16NP)
    dt_w = np.ascontiguousarray(p["dt_w"][:, sl_own].astype(BF16NP))
    dt_b = _to_pblocks(p["dt_b"][sl_own][:, None], NBLK_H, np.float32)
    A = _to_pblocks(-np.exp(p["A_log"][sl_own]), NBLK_H, np.float32)
    D = _to_pblocks(p["D_skip"][sl_own][:, None], NBLK_H, np.float32)

    fuse_half = fuse_w[:D_MODEL] if dr == 0 else fuse_w[D_MODEL:]
    w_out_full = p["out_w"].astype(np.float64) @ fuse_half.astype(np.float64)
    w_out = _to_pblocks(w_out_full[sl_own].astype(np.float32), 4, BF16NP)

    bcsel = np.zeros((N_STATE, N_STATE * 128), BF16NP)
    for n in range(N_STATE):
        bcsel[n, n * 128:(n + 1) * 128] = 1.0

    return {
        "xT": xT, "w_in": w_in, "conv_w": conv_w, "conv_b": conv_b,
        "xproj_w": xproj_w, "dt_w": dt_w, "dt_b": dt_b, "A": A, "D": D,
        "w_out": w_out, "bcsel": bcsel,
    }


def kernel(_spmd_kwargs=None, **inputs):
    hs = np.asarray(inputs["hidden_states"], dtype=np.float32)
    fuse_w = np.asarray(inputs["fuse_w"], dtype=np.float32)
    fuse_b = np.asarray(inputs["fuse_b"], dtype=np.float32)
    params = []
    for pre in ("fwd_", "bwd_"):
        params.append({k[len(pre):]: np.asarray(v, dtype=np.float32)
                       for k, v in inputs.items() if k.startswith(pre)})

    nc = _get_program()

    in_maps = []
    core_cfg = []
    prep_cache = {}
    for c in range(8):
        b, dr, h = c >> 2, (c >> 1) & 1, c & 1
        core_cfg.append((b, dr, h))
        key = (b, dr, h)
        if key not in prep_cache:
            prep_cache[key] = _core_inputs(hs, params, fuse_w, b, dr, h)
        in_maps.append(prep_cache[key])

    res = run_bass_kernel_spmd(nc, in_maps, core_ids=list(range(8)),
                               **(_spmd_kwargs or {}))

    out = np.zeros((B, L, D_MODEL), dtype=np.float32)
    for c in range(8):
        b, dr, h = core_cfg[c]
        contrib = res.results[c]["out_part"].T  # (L, D_MODEL)
        if dr == 1:
            contrib = contrib[::-1]
        out[b] += contrib
    out += fuse_b[None, None, :]
    if _spmd_kwargs is not None:
        kernel._last_result = res
    return out
